# revision 3
# baseline (speedup 1.0000x reference)
"""MoE layer (SwiGLU experts, top-2 routing) on 8 Trainium2 NeuronCores.

Strategy (per the expert-parallel sharding hint):
  Launch A (data-parallel): each core takes 1/8 of the tokens and computes
    the router on-device: logits = x @ router_w.T via an exact bf16 hi/lo
    split (hi@rhi + lo@rhi + hi@rlo; the dropped lo@rlo term is ~5e-6,
    negligible vs the ~2.6e-4 min top-2 logit gap), then top-2 + softmax
    (sigmoid of the logit difference) -> combine weights [SHARD, E].
  Host: builds the dispatch (the "all-to-all"): per-expert token lists,
    gathers + transposes + bf16-casts the routed token rows, pads to a
    common capacity.
  Launch B (expert-parallel): core e owns expert e's weights (bf16). It
    computes silu(x@wg) * (x@wu) @ wd in bf16 on the tensor engine (bf16
    streams the moving operand at 1 elem/lane/cycle vs 1/2 for fp32, so
    matmuls run 2x faster warm), scales rows by the combine weight, and
    writes the result [D, cap] in bf16.
  Host: scatter-adds the per-expert results back into the [B, S, D] output.

kernel() is self-contained: shapes/sharding are hardcoded for
  x[2, 2048, 512], router_w[8, 512], w_gate[8, 512, 1024],
  w_up[8, 512, 1024], w_down[8, 1024, 512].
"""
import numpy as np
import ml_dtypes

import concourse.bass as bass
import concourse.mybir as mybir
import concourse.tile as tile
from concourse import bacc
from concourse.bass_utils import run_bass_kernel_spmd
from concourse.masks import make_identity

P = 128
B, S, D, H, E, TOPK = 2, 2048, 512, 1024, 8, 2
S_TOT = B * S            # 4096 tokens
N_CORES = 8
SHARD = S_TOT // N_CORES  # 512 tokens per core in the router launch
KD = D // P               # 4 k-tiles over D
KH = H // P               # 8 k-tiles over H

F32 = mybir.dt.float32
BF16 = mybir.dt.bfloat16
NP_BF16 = ml_dtypes.bfloat16
AF = mybir.ActivationFunctionType

_router_nc = None
_expert_nc = {}


def _chunks_of(cap):
    """Token chunks of 512 (PSUM-bank width), remainder last (short tail)."""
    out, n0 = [], 0
    while n0 < cap:
        sz = min(512, cap - n0)
        out.append((n0, sz))
        n0 += sz
    return out


def _build_router():
    """Per core: token shard transposed, split hi/lo bf16 -> fp32-exact
    logits -> top-2 sigmoid-softmax combine weights dw [SHARD, E] f32."""
    nc = bacc.Bacc(None, target_bir_lowering=False)
    xthi = nc.dram_tensor("xthi", [D, SHARD], BF16, kind="ExternalInput")
    xtlo = nc.dram_tensor("xtlo", [D, SHARD], BF16, kind="ExternalInput")
    rhi = nc.dram_tensor("rhi", [D, E], BF16, kind="ExternalInput")
    rlo = nc.dram_tensor("rlo", [D, E], BF16, kind="ExternalInput")
    dw = nc.dram_tensor("dw", [SHARD, E], F32, kind="ExternalOutput")

    M4 = SHARD // P  # 4 token groups of 128
    with tile.TileContext(nc) as tc:
        with tc.tile_pool(name="sb", bufs=1) as sb, \
             tc.tile_pool(name="wk", bufs=1) as wk, \
             tc.tile_pool(name="ps", bufs=1, space="PSUM") as ps:
            ident = sb.tile([P, P], F32)
            make_identity(nc, ident[:])

            rhi_t = sb.tile([P, KD, E], BF16)
            rlo_t = sb.tile([P, KD, E], BF16)
            nc.sync.dma_start(out=rhi_t[:], in_=rhi.rearrange("(k p) e -> p k e", p=P))
            nc.sync.dma_start(out=rlo_t[:], in_=rlo.rearrange("(k p) e -> p k e", p=P))
            xthi_t = sb.tile([P, KD, SHARD], BF16)
            xtlo_t = sb.tile([P, KD, SHARD], BF16)
            xthi_r = xthi.rearrange("(k p) n -> p k n", p=P)
            xtlo_r = xtlo.rearrange("(k p) n -> p k n", p=P)
            for k in range(KD):
                nc.sync.dma_start(out=xthi_t[:, k, :], in_=xthi_r[:, k, :])
            for k in range(KD):
                nc.scalar.dma_start(out=xtlo_t[:, k, :], in_=xtlo_r[:, k, :])

            # logitsT [E, SHARD]: 12 bf16 matmuls, one accumulation group
            psl = ps.tile([E, SHARD], F32, tag="psl")
            terms = ([(rhi_t, xthi_t, k) for k in range(KD)]
                     + [(rlo_t, xthi_t, k) for k in range(KD)]
                     + [(rhi_t, xtlo_t, k) for k in range(KD)])
            for i, (r_t, x_t, k) in enumerate(terms):
                nc.tensor.matmul(psl[:], r_t[:, k, :], x_t[:, k, :],
                                 start=(i == 0), stop=(i == len(terms) - 1))
            lgT = sb.tile([E, SHARD], F32)
            nc.vector.tensor_copy(lgT[:], psl[:])

            # transpose to token-major [tok, E] groups
            lg_all = wk.tile([P, M4, E], F32)
            for m in range(M4):
                pst = ps.tile([P, E], F32, tag=f"pst{m}", name=f"pst{m}")
                nc.tensor.transpose(pst[:], lgT[:, m * P:(m + 1) * P], ident[:E, :E])
                nc.vector.tensor_copy(lg_all[:, m, :], pst[:])

            # top-2 softmax over the top-2 logits == sigmoid(+/- logit diff)
            m1 = wk.tile([P, M4, 1], F32)
            nc.vector.tensor_reduce(m1[:], lg_all[:], axis=mybir.AxisListType.X,
                                    op=mybir.AluOpType.max)
            msk1 = wk.tile([P, M4, E], F32)
            nc.vector.tensor_tensor(out=msk1[:], in0=lg_all[:],
                                    in1=m1[:].to_broadcast([P, M4, E]),
                                    op=mybir.AluOpType.is_equal)
            lg2 = wk.tile([P, M4, E], F32)
            nc.vector.scalar_tensor_tensor(
                out=lg2[:], in0=msk1[:], scalar=-1e30, in1=lg_all[:],
                op0=mybir.AluOpType.mult, op1=mybir.AluOpType.add)
            m2 = wk.tile([P, M4, 1], F32)
            nc.vector.tensor_reduce(m2[:], lg2[:], axis=mybir.AxisListType.X,
                                    op=mybir.AluOpType.max)
            d12 = wk.tile([P, M4, 1], F32)
            nc.vector.tensor_sub(d12[:], m1[:], m2[:])
            d21 = wk.tile([P, M4, 1], F32)
            nc.vector.tensor_sub(d21[:], m2[:], m1[:])
            s1 = wk.tile([P, M4, 1], F32)
            nc.scalar.activation(s1[:], d12[:], AF.Sigmoid)
            s2 = wk.tile([P, M4, 1], F32)
            nc.scalar.activation(s2[:], d21[:], AF.Sigmoid)
            msk2 = wk.tile([P, M4, E], F32)
            nc.vector.tensor_tensor(out=msk2[:], in0=lg2[:],
                                    in1=m2[:].to_broadcast([P, M4, E]),
                                    op=mybir.AluOpType.is_equal)
            t1 = wk.tile([P, M4, E], F32)
            nc.vector.tensor_mul(t1[:], msk1[:], s1[:].to_broadcast([P, M4, E]))
            wout = wk.tile([P, M4, E], F32)
            nc.vector.tensor_mul(wout[:], msk2[:], s2[:].to_broadcast([P, M4, E]))
            nc.vector.tensor_add(wout[:], wout[:], t1[:])
            nc.sync.dma_start(out=dw.rearrange("(m p) e -> p m e", p=P), in_=wout[:])
    nc.compile()
    return nc


def _build_expert(cap):
    """Per core (expert e): host-gathered, transposed, bf16 routed tokens
    xgT [D, cap] -> SwiGLU in bf16 -> scale by combine weight -> yt bf16."""
    nc = bacc.Bacc(None, target_bir_lowering=False)
    xgT = nc.dram_tensor("xgT", [D, cap], BF16, kind="ExternalInput")
    wg = nc.dram_tensor("wg", [D, H], BF16, kind="ExternalInput")
    wu = nc.dram_tensor("wu", [D, H], BF16, kind="ExternalInput")
    wd = nc.dram_tensor("wd", [H, D], BF16, kind="ExternalInput")
    wtb = nc.dram_tensor("wtb", [P, cap], F32, kind="ExternalInput")
    yt = nc.dram_tensor("yt", [D, cap], BF16, kind="ExternalOutput")

    chunks = _chunks_of(cap)
    (c0_n0, c0_nsz) = chunks[0]

    with tile.TileContext(nc) as tc:
        with tc.tile_pool(name="wts", bufs=1) as wts, \
             tc.tile_pool(name="ap", bufs=3) as ap, \
             tc.tile_pool(name="ps_gu", bufs=2, space="PSUM") as ps_gu, \
             tc.tile_pool(name="ps_y", bufs=1, space="PSUM") as ps_y:

            # PE warmup fillers: keep the PE busy from the preamble's end so
            # the HAM clock ramps while the first DMAs land.
            warm = wts.tile([P, 384], BF16)
            nc.gpsimd.memset(warm[:], 0.0)
            for w in range(6):
                psw = ps_gu.tile([P, 384], F32, tag="psg", name=f"psw{w}")
                nc.tensor.matmul(psw[:], warm[:, :P], warm[:], start=True, stop=True)

            wg_t = wts.tile([P, KD, H], BF16)
            wu_t = wts.tile([P, KD, H], BF16)
            wd_t = wts.tile([P, KH, D], BF16)
            xgT_t = wts.tile([P, KD, cap], BF16)
            wtb_t = wts.tile([P, cap], F32)
            wg_r = wg.rearrange("(k p) h -> p k h", p=P)
            wu_r = wu.rearrange("(k p) h -> p k h", p=P)
            wd_r = wd.rearrange("(k p) d -> p k d", p=P)
            xgT_r = xgT.rearrange("(k p) n -> p k n", p=P)

            # DMA issue order tracks first use. Sync ring: chunk-0 tokens
            # and the gate/up weight slices; scalar ring: later chunks,
            # wd, wtb.
            c0 = slice(c0_n0, c0_n0 + c0_nsz)
            for k in range(KD):
                nc.sync.dma_start(out=xgT_t[:, k, c0], in_=xgT_r[:, k, c0])
            for h in range(KH):
                hs = slice(h * P, (h + 1) * P)
                nc.sync.dma_start(out=wg_t[:, :, hs], in_=wg_r[:, :, hs])
                nc.sync.dma_start(out=wu_t[:, :, hs], in_=wu_r[:, :, hs])
            for h in range(KH):
                nc.scalar.dma_start(out=wd_t[:, h, :], in_=wd_r[:, h, :])
            for (n0, nsz) in chunks[1:]:
                cs = slice(n0, n0 + nsz)
                for k in range(KD):
                    nc.scalar.dma_start(out=xgT_t[:, k, cs], in_=xgT_r[:, k, cs])
            nc.scalar.dma_start(out=wtb_t[:], in_=wtb[:, :])

            for (n0, nsz) in chunks:
                cs = slice(n0, n0 + nsz)
                psy = [ps_y.tile([P, nsz], F32, tag=f"psy{d}", name=f"psy{d}_{n0}")
                       for d in range(KD)]
                for h in range(KH):
                    psg = ps_gu.tile([P, nsz], F32, tag="psg")
                    psu = ps_gu.tile([P, nsz], F32, tag="psu")
                    for k in range(KD):
                        nc.tensor.matmul(
                            psg[:], wg_t[:, k, h * P:(h + 1) * P], xgT_t[:, k, cs],
                            start=(k == 0), stop=(k == KD - 1))
                    for k in range(KD):
                        nc.tensor.matmul(
                            psu[:], wu_t[:, k, h * P:(h + 1) * P], xgT_t[:, k, cs],
                            start=(k == 0), stop=(k == KD - 1))
                    actg = ap.tile([P, nsz], F32, tag="actg")
                    nc.scalar.activation(actg[:], psg[:], AF.Silu)
                    act = ap.tile([P, nsz], BF16, tag="act")
                    nc.vector.tensor_mul(act[:], actg[:], psu[:])
                    for d in range(KD):
                        nc.tensor.matmul(
                            psy[d][:], wd_t[:, h, d * P:(d + 1) * P], act[:],
                            start=(h == 0), stop=(h == KH - 1))
                for d in range(KD):
                    yts = ap.tile([P, nsz], BF16, tag="yts")
                    nc.vector.tensor_mul(yts[:], psy[d][:], wtb_t[:, cs])
                    nc.sync.dma_start(out=yt[d * P:(d + 1) * P, cs], in_=yts[:])
    nc.compile()
    return nc


def _get_router_nc():
    global _router_nc
    if _router_nc is None:
        _router_nc = _build_router()
    return _router_nc


def _get_expert_nc(cap):
    if cap not in _expert_nc:
        _expert_nc[cap] = _build_expert(cap)
    return _expert_nc[cap]


def kernel(x, router_w, w_gate, w_up, w_down, _timings=None):
    x = np.ascontiguousarray(x, dtype=np.float32)
    router_w = np.ascontiguousarray(router_w, dtype=np.float32)

    flat = x.reshape(S_TOT, D)
    rwt = np.ascontiguousarray(router_w.T)  # [D, E]
    rhi = rwt.astype(NP_BF16)
    rlo = (rwt - rhi.astype(np.float32)).astype(NP_BF16)

    # ---- Launch A: router (data-parallel over token shards) ----
    nc_a = _get_router_nc()
    in_maps_a = []
    for c in range(N_CORES):
        sh = np.ascontiguousarray(flat[c * SHARD:(c + 1) * SHARD].T)  # [D, SHARD]
        xthi = sh.astype(NP_BF16)
        xtlo = (sh - xthi.astype(np.float32)).astype(NP_BF16)
        in_maps_a.append({"xthi": xthi, "xtlo": xtlo, "rhi": rhi, "rlo": rlo})
    res_a = run_bass_kernel_spmd(nc_a, in_maps_a, core_ids=list(range(N_CORES)))
    dw = np.concatenate([res_a.results[c]["dw"] for c in range(N_CORES)], axis=0)
    if _timings is not None:
        _timings["router_ns"] = res_a.exec_time_ns

    # ---- Host: build the dispatch (the all-to-all by expert) ----
    sel = dw > 0.0
    idx_list = [np.nonzero(sel[:, e])[0].astype(np.int32) for e in range(E)]
    counts = [len(ix) for ix in idx_list]
    cap = max(max(counts), 1)
    cap = ((cap + P - 1) // P) * P

    flat_bf = flat.astype(NP_BF16)
    wg_bf = np.asarray(w_gate, dtype=NP_BF16)
    wu_bf = np.asarray(w_up, dtype=NP_BF16)
    wd_bf = np.asarray(w_down, dtype=NP_BF16)

    in_maps_b = []
    for e in range(E):
        ix = idx_list[e]
        xg = np.zeros((cap, D), dtype=NP_BF16)
        xg[:len(ix)] = flat_bf[ix]
        wt = np.zeros(cap, dtype=np.float32)
        wt[:len(ix)] = dw[ix, e]
        in_maps_b.append({
            "xgT": np.ascontiguousarray(xg.T),
            "wg": np.ascontiguousarray(wg_bf[e]),
            "wu": np.ascontiguousarray(wu_bf[e]),
            "wd": np.ascontiguousarray(wd_bf[e]),
            "wtb": np.ascontiguousarray(np.broadcast_to(wt[None, :], (P, cap))),
        })

    # ---- Launch B: experts (expert-parallel) ----
    nc_b = _get_expert_nc(cap)
    res_b = run_bass_kernel_spmd(nc_b, in_maps_b, core_ids=list(range(N_CORES)))
    if _timings is not None:
        _timings["expert_ns"] = res_b.exec_time_ns

    # ---- Host: combine (scatter-add back, then unshard) ----
    out = np.zeros((S_TOT, D), dtype=np.float32)
    for e in range(E):
        ix = idx_list[e]
        if len(ix) == 0:
            continue
        ytc = res_b.results[e]["yt"][:, :len(ix)].astype(np.float32)
        out[ix] += ytc.T  # indices unique per expert
    return out.reshape(B, S, D)


# revision 5
# speedup vs baseline: 1.0469x; 1.0469x over previous
"""MoE layer (SwiGLU experts, top-2 routing) on 8 Trainium2 NeuronCores.

Strategy (per the expert-parallel sharding hint):
  Launch A (data-parallel): each core takes 1/8 of the tokens and computes
    the router on-device: logits = x @ router_w.T via an exact bf16 hi/lo
    split (hi@rhi + lo@rhi + hi@rlo; the dropped lo@rlo term is ~5e-6,
    negligible vs the ~2.6e-4 min top-2 logit gap), then top-2 + softmax
    (sigmoid of the logit difference) -> combine weights [SHARD, E].
  Host: builds the dispatch (the "all-to-all"): per-expert token lists,
    gathers + transposes + bf16-casts the routed token rows, pads to a
    common capacity.
  Launch B (expert-parallel): core e owns expert e's weights (bf16). It
    computes silu(x@wg) * (x@wu) @ wd in bf16 on the tensor engine (bf16
    streams the moving operand at 1 elem/lane/cycle vs 1/2 for fp32, so
    matmuls run 2x faster warm), scales rows by the combine weight, and
    writes the result in bf16.
  Host: scatter-adds the per-expert results back into the [B, S, D] output.

All DRAM tensors are host-packed into their exact SBUF layouts
([128 partitions, ...] with >=1KB contiguous per-partition pieces) so each
transfer is a single cheap-to-issue DMA: dma_start issue cost is ~0.7us
per instruction on the issuing engine and small descriptors kill both
issue time and bandwidth. DMAs are spread across the sync + scalar HWDGE
rings and the gpsimd SWDGE ring.

kernel() is self-contained: shapes/sharding are hardcoded for
  x[2, 2048, 512], router_w[8, 512], w_gate[8, 512, 1024],
  w_up[8, 512, 1024], w_down[8, 1024, 512].
"""
import numpy as np
import ml_dtypes

import concourse.bass as bass
import concourse.mybir as mybir
import concourse.tile as tile
from concourse import bacc
from concourse.bass_utils import run_bass_kernel_spmd
from concourse.masks import make_identity

P = 128
B, S, D, H, E, TOPK = 2, 2048, 512, 1024, 8, 2
S_TOT = B * S            # 4096 tokens
N_CORES = 8
SHARD = S_TOT // N_CORES  # 512 tokens per core in the router launch
KD = D // P               # 4 k-tiles over D
KH = H // P               # 8 k-tiles over H

F32 = mybir.dt.float32
BF16 = mybir.dt.bfloat16
NP_BF16 = ml_dtypes.bfloat16
AF = mybir.ActivationFunctionType

_router_nc = None
_expert_nc = {}


def _chunks_of(cap):
    """Token chunks of 512 (PSUM-bank width), remainder last (short tail)."""
    out, n0 = [], 0
    while n0 < cap:
        sz = min(512, cap - n0)
        out.append((n0, sz))
        n0 += sz
    return out


def _pack(a, kp):
    """[K*P, N] row-major -> SBUF layout [P, K, N] (partition-major)."""
    k = a.shape[0] // kp
    return np.ascontiguousarray(a.reshape(k, kp, -1).transpose(1, 0, 2))


def _build_router():
    """Per core: token shard transposed, split hi/lo bf16 -> fp32-exact
    logits -> top-2 sigmoid-softmax combine weights dwp [P, M4, E] f32."""
    nc = bacc.Bacc(None, target_bir_lowering=False)
    M4 = SHARD // P  # 4 token groups of 128
    xthi = nc.dram_tensor("xthi", [P, KD, SHARD], BF16, kind="ExternalInput")
    xtlo = nc.dram_tensor("xtlo", [P, KD, SHARD], BF16, kind="ExternalInput")
    rws = nc.dram_tensor("rws", [P, 2, KD, E], BF16, kind="ExternalInput")
    dwp = nc.dram_tensor("dwp", [P, M4, E], F32, kind="ExternalOutput")

    H2 = SHARD // 2
    with tile.TileContext(nc) as tc:
        with tc.tile_pool(name="sb", bufs=1) as sb, \
             tc.tile_pool(name="wk", bufs=1) as wk, \
             tc.tile_pool(name="ps", bufs=1, space="PSUM") as ps:
            ident = sb.tile([P, P], F32)
            make_identity(nc, ident[:])
            warm = sb.tile([P, 256], BF16)
            nc.gpsimd.memset(warm[:], 0.0)

            r_t = sb.tile([P, 2, KD, E], BF16)
            xthi_t = sb.tile([P, KD, SHARD], BF16)
            xtlo_t = sb.tile([P, KD, SHARD], BF16)
            nc.sync.dma_start(out=r_t[:], in_=rws[:, :, :, :])
            nc.sync.dma_start(out=xthi_t[:, :, :H2], in_=xthi[:, :, :H2])
            nc.sync.dma_start(out=xthi_t[:, :, H2:], in_=xthi[:, :, H2:])
            nc.scalar.dma_start(out=xtlo_t[:, :, :H2], in_=xtlo[:, :, :H2])
            nc.scalar.dma_start(out=xtlo_t[:, :, H2:], in_=xtlo[:, :, H2:])

            # PE warmup fillers bridge until the first operands land
            for w in range(5):
                psw = ps.tile([P, 256], F32, tag="psw", name=f"psw{w}")
                nc.tensor.matmul(psw[:], warm[:, :P], warm[:], start=True, stop=True)

            # logitsT [E, SHARD]: 12 bf16 matmuls, one accumulation group;
            # xtlo is only needed from term 9 on.
            psl = ps.tile([E, SHARD], F32, tag="psl")
            terms = ([(0, xthi_t, k) for k in range(KD)]
                     + [(1, xthi_t, k) for k in range(KD)]
                     + [(0, xtlo_t, k) for k in range(KD)])
            for i, (s, x_t, k) in enumerate(terms):
                nc.tensor.matmul(psl[:], r_t[:, s, k, :], x_t[:, k, :],
                                 start=(i == 0), stop=(i == len(terms) - 1))
            lgT = sb.tile([E, SHARD], F32)
            nc.vector.tensor_copy(lgT[:], psl[:])

            # transpose to token-major [tok, E] groups
            lg_all = wk.tile([P, M4, E], F32)
            for m in range(M4):
                pst = ps.tile([P, E], F32, tag=f"pst{m}", name=f"pst{m}")
                nc.tensor.transpose(pst[:], lgT[:, m * P:(m + 1) * P], ident[:E, :E])
                nc.vector.tensor_copy(lg_all[:, m, :], pst[:])

            # top-2 softmax over the top-2 logits == sigmoid(+/- logit diff)
            m1 = wk.tile([P, M4, 1], F32)
            nc.vector.tensor_reduce(m1[:], lg_all[:], axis=mybir.AxisListType.X,
                                    op=mybir.AluOpType.max)
            msk1 = wk.tile([P, M4, E], F32)
            nc.vector.tensor_tensor(out=msk1[:], in0=lg_all[:],
                                    in1=m1[:].to_broadcast([P, M4, E]),
                                    op=mybir.AluOpType.is_equal)
            lg2 = wk.tile([P, M4, E], F32)
            nc.vector.scalar_tensor_tensor(
                out=lg2[:], in0=msk1[:], scalar=-1e30, in1=lg_all[:],
                op0=mybir.AluOpType.mult, op1=mybir.AluOpType.add)
            m2 = wk.tile([P, M4, 1], F32)
            nc.vector.tensor_reduce(m2[:], lg2[:], axis=mybir.AxisListType.X,
                                    op=mybir.AluOpType.max)
            d12 = wk.tile([P, M4, 1], F32)
            nc.vector.tensor_sub(d12[:], m1[:], m2[:])
            d21 = wk.tile([P, M4, 1], F32)
            nc.vector.tensor_sub(d21[:], m2[:], m1[:])
            s1 = wk.tile([P, M4, 1], F32)
            nc.scalar.activation(s1[:], d12[:], AF.Sigmoid)
            s2 = wk.tile([P, M4, 1], F32)
            nc.scalar.activation(s2[:], d21[:], AF.Sigmoid)
            msk2 = wk.tile([P, M4, E], F32)
            nc.vector.tensor_tensor(out=msk2[:], in0=lg2[:],
                                    in1=m2[:].to_broadcast([P, M4, E]),
                                    op=mybir.AluOpType.is_equal)
            t1 = wk.tile([P, M4, E], F32)
            nc.vector.tensor_mul(t1[:], msk1[:], s1[:].to_broadcast([P, M4, E]))
            wout = wk.tile([P, M4, E], F32)
            nc.vector.tensor_mul(wout[:], msk2[:], s2[:].to_broadcast([P, M4, E]))
            nc.vector.tensor_add(wout[:], wout[:], t1[:])
            nc.sync.dma_start(out=dwp[:, :, :], in_=wout[:])
    nc.compile()
    return nc


def _build_expert(cap):
    """Per core (expert e): host-gathered, transposed, bf16 routed tokens
    xgT [P, KD, cap] -> SwiGLU in bf16 -> scale by combine weight -> yt bf16."""
    nc = bacc.Bacc(None, target_bir_lowering=False)
    xgT = nc.dram_tensor("xgT", [P, KD, cap], BF16, kind="ExternalInput")
    wg = nc.dram_tensor("wg", [P, KD, H], BF16, kind="ExternalInput")
    wu = nc.dram_tensor("wu", [P, KD, H], BF16, kind="ExternalInput")
    wd = nc.dram_tensor("wd", [P, KH, D], BF16, kind="ExternalInput")
    wtb = nc.dram_tensor("wtb", [P, cap], F32, kind="ExternalInput")
    yt = nc.dram_tensor("yt", [P, KD, cap], BF16, kind="ExternalOutput")

    chunks = _chunks_of(cap)
    (c0_n0, c0_nsz) = chunks[0]
    c0 = slice(c0_n0, c0_n0 + c0_nsz)
    H2 = H // 2

    with tile.TileContext(nc) as tc:
        with tc.tile_pool(name="wts", bufs=1) as wts, \
             tc.tile_pool(name="ap", bufs=3) as ap, \
             tc.tile_pool(name="ps_gu", bufs=2, space="PSUM") as ps_gu, \
             tc.tile_pool(name="ps_y", bufs=1, space="PSUM") as ps_y:

            # PE warmup fillers: keep the PE busy from the preamble's end so
            # the HAM clock ramps while the first DMAs land.
            warm = wts.tile([P, 384], BF16)
            nc.gpsimd.memset(warm[:], 0.0)
            for w in range(7):
                psw = ps_gu.tile([P, 384], F32, tag="psg", name=f"psw{w}")
                nc.tensor.matmul(psw[:], warm[:, :P], warm[:], start=True, stop=True)

            wg_t = wts.tile([P, KD, H], BF16)
            wu_t = wts.tile([P, KD, H], BF16)
            wd_t = wts.tile([P, KH, D], BF16)
            xgT_t = wts.tile([P, KD, cap], BF16)
            wtb_t = wts.tile([P, cap], F32)

            # DMA order tracks first use; two HWDGE rings + gpsimd SWDGE.
            nc.sync.dma_start(out=xgT_t[:, :, c0], in_=xgT[:, :, c0])
            nc.scalar.dma_start(out=wg_t[:, :, :H2], in_=wg[:, :, :H2])
            nc.sync.dma_start(out=wu_t[:, :, :H2], in_=wu[:, :, :H2])
            nc.scalar.dma_start(out=wd_t[:, :KH // 2, :], in_=wd[:, :KH // 2, :])
            nc.sync.dma_start(out=wg_t[:, :, H2:], in_=wg[:, :, H2:])
            nc.scalar.dma_start(out=wd_t[:, KH // 2:, :], in_=wd[:, KH // 2:, :])
            nc.sync.dma_start(out=wu_t[:, :, H2:], in_=wu[:, :, H2:])
            for (n0, nsz) in chunks[1:]:
                cs = slice(n0, n0 + nsz)
                nc.scalar.dma_start(out=xgT_t[:, :, cs], in_=xgT[:, :, cs])
            nc.gpsimd.dma_start(out=wtb_t[:], in_=wtb[:, :])

            for (n0, nsz) in chunks:
                cs = slice(n0, n0 + nsz)
                psy = [ps_y.tile([P, nsz], F32, tag=f"psy{d}", name=f"psy{d}_{n0}")
                       for d in range(KD)]
                for h in range(KH):
                    psg = ps_gu.tile([P, nsz], F32, tag="psg")
                    psu = ps_gu.tile([P, nsz], F32, tag="psu")
                    for k in range(KD):
                        nc.tensor.matmul(
                            psg[:], wg_t[:, k, h * P:(h + 1) * P], xgT_t[:, k, cs],
                            start=(k == 0), stop=(k == KD - 1))
                    for k in range(KD):
                        nc.tensor.matmul(
                            psu[:], wu_t[:, k, h * P:(h + 1) * P], xgT_t[:, k, cs],
                            start=(k == 0), stop=(k == KD - 1))
                    actg = ap.tile([P, nsz], F32, tag="actg")
                    nc.scalar.activation(actg[:], psg[:], AF.Silu)
                    act = ap.tile([P, nsz], BF16, tag="act")
                    nc.vector.tensor_mul(act[:], actg[:], psu[:])
                    for d in range(KD):
                        nc.tensor.matmul(
                            psy[d][:], wd_t[:, h, d * P:(d + 1) * P], act[:],
                            start=(h == 0), stop=(h == KH - 1))
                yts = ap.tile([P, KD, nsz], BF16, tag="yts")
                for d in range(KD):
                    nc.vector.tensor_mul(yts[:, d, :], psy[d][:], wtb_t[:, cs])
                nc.sync.dma_start(out=yt[:, :, cs], in_=yts[:])
    nc.compile()
    return nc


def _get_router_nc():
    global _router_nc
    if _router_nc is None:
        _router_nc = _build_router()
    return _router_nc


def _get_expert_nc(cap):
    if cap not in _expert_nc:
        _expert_nc[cap] = _build_expert(cap)
    return _expert_nc[cap]


def kernel(x, router_w, w_gate, w_up, w_down, _timings=None):
    x = np.ascontiguousarray(x, dtype=np.float32)
    router_w = np.ascontiguousarray(router_w, dtype=np.float32)

    flat = x.reshape(S_TOT, D)
    rwt = np.ascontiguousarray(router_w.T)  # [D, E]
    rhi = rwt.astype(NP_BF16)
    rlo = (rwt - rhi.astype(np.float32)).astype(NP_BF16)
    # pack router weights into SBUF layout [P, 2, KD, E]
    rws = np.stack([_pack(rhi, P), _pack(rlo, P)], axis=1)
    rws = np.ascontiguousarray(rws)

    # ---- Launch A: router (data-parallel over token shards) ----
    nc_a = _get_router_nc()
    M4 = SHARD // P
    in_maps_a = []
    for c in range(N_CORES):
        sh = np.ascontiguousarray(flat[c * SHARD:(c + 1) * SHARD].T)  # [D, SHARD]
        xthi = sh.astype(NP_BF16)
        xtlo = (sh - xthi.astype(np.float32)).astype(NP_BF16)
        in_maps_a.append({"xthi": _pack(xthi, P), "xtlo": _pack(xtlo, P),
                          "rws": rws})
    res_a = run_bass_kernel_spmd(nc_a, in_maps_a, core_ids=list(range(N_CORES)))
    # unpack [P, M4, E] -> [SHARD, E] per core, concat to [S_TOT, E]
    dw = np.concatenate(
        [res_a.results[c]["dwp"].transpose(1, 0, 2).reshape(SHARD, E)
         for c in range(N_CORES)], axis=0)
    if _timings is not None:
        _timings["router_ns"] = res_a.exec_time_ns

    # ---- Host: build the dispatch (the all-to-all by expert) ----
    sel = dw > 0.0
    idx_list = [np.nonzero(sel[:, e])[0].astype(np.int32) for e in range(E)]
    counts = [len(ix) for ix in idx_list]
    cap = max(max(counts), 1)
    cap = ((cap + P - 1) // P) * P

    flat_bf = flat.astype(NP_BF16)
    wg_bf = np.asarray(w_gate, dtype=NP_BF16)
    wu_bf = np.asarray(w_up, dtype=NP_BF16)
    wd_bf = np.asarray(w_down, dtype=NP_BF16)

    in_maps_b = []
    for e in range(E):
        ix = idx_list[e]
        xg = np.zeros((cap, D), dtype=NP_BF16)
        xg[:len(ix)] = flat_bf[ix]
        wt = np.zeros(cap, dtype=np.float32)
        wt[:len(ix)] = dw[ix, e]
        in_maps_b.append({
            "xgT": _pack(np.ascontiguousarray(xg.T), P),
            "wg": _pack(wg_bf[e], P),
            "wu": _pack(wu_bf[e], P),
            "wd": _pack(wd_bf[e], P),
            "wtb": np.ascontiguousarray(np.broadcast_to(wt[None, :], (P, cap))),
        })

    # ---- Launch B: experts (expert-parallel) ----
    nc_b = _get_expert_nc(cap)
    res_b = run_bass_kernel_spmd(nc_b, in_maps_b, core_ids=list(range(N_CORES)))
    if _timings is not None:
        _timings["expert_ns"] = res_b.exec_time_ns

    # ---- Host: combine (scatter-add back, then unshard) ----
    out = np.zeros((S_TOT, D), dtype=np.float32)
    for e in range(E):
        ix = idx_list[e]
        if len(ix) == 0:
            continue
        ytp = res_b.results[e]["yt"]  # [P, KD, cap] bf16
        ytc = ytp.transpose(1, 0, 2).reshape(D, cap)[:, :len(ix)].astype(np.float32)
        out[ix] += ytc.T  # indices unique per expert
    return out.reshape(B, S, D)


# revision 7
# speedup vs baseline: 1.1050x; 1.0555x over previous
"""MoE layer (SwiGLU experts, top-2 routing) on 8 Trainium2 NeuronCores.

Strategy (per the expert-parallel sharding hint):
  Launch A (data-parallel): each core takes 1/8 of the tokens and computes
    the router on-device: logits = x @ router_w.T via an exact bf16 hi/lo
    split (hi@rhi + lo@rhi + hi@rlo; the dropped lo@rlo term is ~5e-6,
    negligible vs the ~2.6e-4 min top-2 logit gap), then top-2 + softmax
    (sigmoid of the logit difference) -> combine weights [SHARD, E].
  Host: builds the dispatch (the "all-to-all"): per-expert token lists,
    gathers + transposes + bf16-casts the routed token rows, pads to a
    common capacity.
  Launch B (expert-parallel): core e owns expert e's weights (bf16). It
    computes silu(x@wg) * (x@wu) @ wd in bf16 on the tensor engine (bf16
    streams the moving operand at 1 elem/lane/cycle vs 1/2 for fp32, so
    matmuls run 2x faster warm), scales rows by the combine weight, and
    writes the result in bf16.
  Host: scatter-adds the per-expert results back into the [B, S, D] output.

All DRAM tensors are host-packed into their exact SBUF layouts
([128 partitions, ...] with >=1KB contiguous per-partition pieces) so each
transfer is a single cheap-to-issue DMA: dma_start issue cost is ~0.7us
per instruction on the issuing engine and small descriptors kill both
issue time and bandwidth. DMAs are spread across the sync + scalar HWDGE
rings and the gpsimd SWDGE ring.

kernel() is self-contained: shapes/sharding are hardcoded for
  x[2, 2048, 512], router_w[8, 512], w_gate[8, 512, 1024],
  w_up[8, 512, 1024], w_down[8, 1024, 512].
"""
import numpy as np
import ml_dtypes

import concourse.bass as bass
import concourse.mybir as mybir
import concourse.tile as tile
from concourse import bacc
from concourse.bass_utils import run_bass_kernel_spmd
from concourse.masks import make_identity

P = 128
B, S, D, H, E, TOPK = 2, 2048, 512, 1024, 8, 2
S_TOT = B * S            # 4096 tokens
N_CORES = 8
SHARD = S_TOT // N_CORES  # 512 tokens per core in the router launch
KD = D // P               # 4 k-tiles over D
KH = H // P               # 8 k-tiles over H

F32 = mybir.dt.float32
BF16 = mybir.dt.bfloat16
NP_BF16 = ml_dtypes.bfloat16
AF = mybir.ActivationFunctionType

_router_nc = None
_expert_nc = {}


def _chunks_of(cap):
    """Token chunks of 512 (PSUM-bank width), remainder last (short tail)."""
    out, n0 = [], 0
    while n0 < cap:
        sz = min(512, cap - n0)
        out.append((n0, sz))
        n0 += sz
    return out


def _pack(a, kp):
    """[K*P, N] row-major -> SBUF layout [P, K, N] (partition-major)."""
    k = a.shape[0] // kp
    return np.ascontiguousarray(a.reshape(k, kp, -1).transpose(1, 0, 2))


def _build_router():
    """Per core: token shard transposed, split hi/lo bf16 -> fp32-exact
    logits -> top-2 sigmoid-softmax combine weights dwp [P, M4, E] f32."""
    nc = bacc.Bacc(None, target_bir_lowering=False)
    M4 = SHARD // P  # 4 token groups of 128
    xthi = nc.dram_tensor("xthi", [P, KD, SHARD], BF16, kind="ExternalInput")
    xtlo = nc.dram_tensor("xtlo", [P, KD, SHARD], BF16, kind="ExternalInput")
    rws = nc.dram_tensor("rws", [P, 2, KD, E], BF16, kind="ExternalInput")
    dwp = nc.dram_tensor("dwp", [P, M4, E], F32, kind="ExternalOutput")

    H2 = SHARD // 2
    with tile.TileContext(nc) as tc:
        with tc.tile_pool(name="sb", bufs=1) as sb, \
             tc.tile_pool(name="wk", bufs=1) as wk, \
             tc.tile_pool(name="ps", bufs=1, space="PSUM") as ps:
            ident = sb.tile([P, P], F32)
            make_identity(nc, ident[:])
            warm = sb.tile([P, 256], BF16)
            nc.gpsimd.memset(warm[:], 0.0)

            r_t = sb.tile([P, 2, KD, E], BF16)
            xthi_t = sb.tile([P, KD, SHARD], BF16)
            xtlo_t = sb.tile([P, KD, SHARD], BF16)
            # k-sliced DMAs match the matmuls' k-tile reads so the logits
            # stream starts on the first slice
            nc.sync.dma_start(out=r_t[:], in_=rws[:, :, :, :])
            for k in range(KD):
                nc.sync.dma_start(out=xthi_t[:, k, :], in_=xthi[:, k, :])
            for k in range(KD):
                nc.scalar.dma_start(out=xtlo_t[:, k, :], in_=xtlo[:, k, :])

            # PE warmup fillers bridge until the first operands land
            for w in range(6):
                psw = ps.tile([P, 256], F32, tag="psw", name=f"psw{w}")
                nc.tensor.matmul(psw[:], warm[:, :P], warm[:], start=True, stop=True)

            # logitsT [E, SHARD]: 12 bf16 matmuls, one accumulation group;
            # k-paced, xtlo only needed from term 9 on.
            psl = ps.tile([E, SHARD], F32, tag="psl")
            terms = ([t for k in range(KD) for t in ((0, xthi_t, k), (1, xthi_t, k))]
                     + [(0, xtlo_t, k) for k in range(KD)])
            for i, (s, x_t, k) in enumerate(terms):
                nc.tensor.matmul(psl[:], r_t[:, s, k, :], x_t[:, k, :],
                                 start=(i == 0), stop=(i == len(terms) - 1))
            lgT = sb.tile([E, SHARD], F32)
            nc.vector.tensor_copy(lgT[:], psl[:])

            # transpose to token-major [tok, E] groups
            lg_all = wk.tile([P, M4, E], F32)
            for m in range(M4):
                pst = ps.tile([P, E], F32, tag=f"pst{m}", name=f"pst{m}")
                nc.tensor.transpose(pst[:], lgT[:, m * P:(m + 1) * P], ident[:E, :E])
                nc.vector.tensor_copy(lg_all[:, m, :], pst[:])

            # top-2 softmax over the top-2 logits == sigmoid(+/- logit diff)
            m1 = wk.tile([P, M4, 1], F32)
            nc.vector.tensor_reduce(m1[:], lg_all[:], axis=mybir.AxisListType.X,
                                    op=mybir.AluOpType.max)
            msk1 = wk.tile([P, M4, E], F32)
            nc.vector.tensor_tensor(out=msk1[:], in0=lg_all[:],
                                    in1=m1[:].to_broadcast([P, M4, E]),
                                    op=mybir.AluOpType.is_equal)
            lg2 = wk.tile([P, M4, E], F32)
            nc.vector.scalar_tensor_tensor(
                out=lg2[:], in0=msk1[:], scalar=-1e30, in1=lg_all[:],
                op0=mybir.AluOpType.mult, op1=mybir.AluOpType.add)
            m2 = wk.tile([P, M4, 1], F32)
            nc.vector.tensor_reduce(m2[:], lg2[:], axis=mybir.AxisListType.X,
                                    op=mybir.AluOpType.max)
            d12 = wk.tile([P, M4, 1], F32)
            nc.vector.tensor_sub(d12[:], m1[:], m2[:])
            d21 = wk.tile([P, M4, 1], F32)
            nc.vector.tensor_sub(d21[:], m2[:], m1[:])
            s1 = wk.tile([P, M4, 1], F32)
            nc.scalar.activation(s1[:], d12[:], AF.Sigmoid)
            s2 = wk.tile([P, M4, 1], F32)
            nc.scalar.activation(s2[:], d21[:], AF.Sigmoid)
            msk2 = wk.tile([P, M4, E], F32)
            nc.vector.tensor_tensor(out=msk2[:], in0=lg2[:],
                                    in1=m2[:].to_broadcast([P, M4, E]),
                                    op=mybir.AluOpType.is_equal)
            t1 = wk.tile([P, M4, E], F32)
            nc.vector.tensor_mul(t1[:], msk1[:], s1[:].to_broadcast([P, M4, E]))
            wout = wk.tile([P, M4, E], F32)
            nc.vector.tensor_mul(wout[:], msk2[:], s2[:].to_broadcast([P, M4, E]))
            nc.vector.tensor_add(wout[:], wout[:], t1[:])
            nc.sync.dma_start(out=dwp[:, :, :], in_=wout[:])
    nc.compile()
    return nc


def _build_expert(cap):
    """Per core (expert e): host-gathered, transposed, bf16 routed tokens
    xgT [P, KD, cap] -> SwiGLU in bf16 -> scale by combine weight -> yt bf16."""
    nc = bacc.Bacc(None, target_bir_lowering=False)
    xgT = nc.dram_tensor("xgT", [P, KD, cap], BF16, kind="ExternalInput")
    wg = nc.dram_tensor("wg", [P, KD, H], BF16, kind="ExternalInput")
    wu = nc.dram_tensor("wu", [P, KD, H], BF16, kind="ExternalInput")
    wd = nc.dram_tensor("wd", [P, KH, D], BF16, kind="ExternalInput")
    wtb = nc.dram_tensor("wtb", [P, cap], F32, kind="ExternalInput")
    yt = nc.dram_tensor("yt", [P, KD, cap], BF16, kind="ExternalOutput")

    chunks = _chunks_of(cap)
    (c0_n0, c0_nsz) = chunks[0]
    c0 = slice(c0_n0, c0_n0 + c0_nsz)
    H2 = H // 2

    with tile.TileContext(nc) as tc:
        with tc.tile_pool(name="wts", bufs=1) as wts, \
             tc.tile_pool(name="ap", bufs=3) as ap, \
             tc.tile_pool(name="ps_gu", bufs=2, space="PSUM") as ps_gu, \
             tc.tile_pool(name="ps_y", bufs=1, space="PSUM") as ps_y:

            # PE warmup fillers: keep the PE busy from the preamble's end so
            # the HAM clock ramps while the first DMAs land.
            warm = wts.tile([P, 384], BF16)
            nc.gpsimd.memset(warm[:], 0.0)
            for w in range(7):
                psw = ps_gu.tile([P, 384], F32, tag="psg", name=f"psw{w}")
                nc.tensor.matmul(psw[:], warm[:, :P], warm[:], start=True, stop=True)

            wg_t = wts.tile([P, KD, H], BF16)
            wu_t = wts.tile([P, KD, H], BF16)
            wd_t = wts.tile([P, KH, D], BF16)
            xgT_t = wts.tile([P, KD, cap], BF16)
            wtb_t = wts.tile([P, cap], F32)

            # DMA order tracks first use across the two HWDGE rings; the
            # first-consumed pieces are small so the MM stream starts early.
            H4 = H // 4
            nc.sync.dma_start(out=xgT_t[:, 0:2, c0], in_=xgT[:, 0:2, c0])
            nc.scalar.dma_start(out=wg_t[:, :, :H4], in_=wg[:, :, :H4])
            nc.sync.dma_start(out=xgT_t[:, 2:4, c0], in_=xgT[:, 2:4, c0])
            nc.scalar.dma_start(out=wu_t[:, :, :H4], in_=wu[:, :, :H4])
            nc.sync.dma_start(out=wg_t[:, :, H4:H2], in_=wg[:, :, H4:H2])
            nc.scalar.dma_start(out=wd_t[:, :KH // 2, :], in_=wd[:, :KH // 2, :])
            nc.sync.dma_start(out=wu_t[:, :, H4:H2], in_=wu[:, :, H4:H2])
            nc.scalar.dma_start(out=wg_t[:, :, H2:], in_=wg[:, :, H2:])
            nc.sync.dma_start(out=wu_t[:, :, H2:], in_=wu[:, :, H2:])
            nc.scalar.dma_start(out=wd_t[:, KH // 2:, :], in_=wd[:, KH // 2:, :])
            for (n0, nsz) in chunks[1:]:
                cs = slice(n0, n0 + nsz)
                nc.scalar.dma_start(out=xgT_t[:, :, cs], in_=xgT[:, :, cs])
            nc.scalar.dma_start(out=wtb_t[:], in_=wtb[:, :])

            for (n0, nsz) in chunks:
                cs = slice(n0, n0 + nsz)
                psy = [ps_y.tile([P, nsz], F32, tag=f"psy{d}", name=f"psy{d}_{n0}")
                       for d in range(KD)]
                for h in range(KH):
                    psg = ps_gu.tile([P, nsz], F32, tag="psg")
                    psu = ps_gu.tile([P, nsz], F32, tag="psu")
                    for k in range(KD):
                        nc.tensor.matmul(
                            psg[:], wg_t[:, k, h * P:(h + 1) * P], xgT_t[:, k, cs],
                            start=(k == 0), stop=(k == KD - 1))
                    for k in range(KD):
                        nc.tensor.matmul(
                            psu[:], wu_t[:, k, h * P:(h + 1) * P], xgT_t[:, k, cs],
                            start=(k == 0), stop=(k == KD - 1))
                    actg = ap.tile([P, nsz], F32, tag="actg")
                    nc.scalar.activation(actg[:], psg[:], AF.Silu)
                    act = ap.tile([P, nsz], BF16, tag="act")
                    nc.vector.tensor_mul(act[:], actg[:], psu[:])
                    for d in range(KD):
                        nc.tensor.matmul(
                            psy[d][:], wd_t[:, h, d * P:(d + 1) * P], act[:],
                            start=(h == 0), stop=(h == KH - 1))
                yts = ap.tile([P, KD, nsz], BF16, tag="yts")
                for d in range(KD):
                    nc.vector.tensor_mul(yts[:, d, :], psy[d][:], wtb_t[:, cs])
                nc.sync.dma_start(out=yt[:, :, cs], in_=yts[:])
    nc.compile()
    return nc


def _get_router_nc():
    global _router_nc
    if _router_nc is None:
        _router_nc = _build_router()
    return _router_nc


def _get_expert_nc(cap):
    if cap not in _expert_nc:
        _expert_nc[cap] = _build_expert(cap)
    return _expert_nc[cap]


def kernel(x, router_w, w_gate, w_up, w_down, _timings=None):
    x = np.ascontiguousarray(x, dtype=np.float32)
    router_w = np.ascontiguousarray(router_w, dtype=np.float32)

    flat = x.reshape(S_TOT, D)
    rwt = np.ascontiguousarray(router_w.T)  # [D, E]
    rhi = rwt.astype(NP_BF16)
    rlo = (rwt - rhi.astype(np.float32)).astype(NP_BF16)
    # pack router weights into SBUF layout [P, 2, KD, E]
    rws = np.stack([_pack(rhi, P), _pack(rlo, P)], axis=1)
    rws = np.ascontiguousarray(rws)

    # ---- Launch A: router (data-parallel over token shards) ----
    nc_a = _get_router_nc()
    M4 = SHARD // P
    in_maps_a = []
    for c in range(N_CORES):
        sh = np.ascontiguousarray(flat[c * SHARD:(c + 1) * SHARD].T)  # [D, SHARD]
        xthi = sh.astype(NP_BF16)
        xtlo = (sh - xthi.astype(np.float32)).astype(NP_BF16)
        in_maps_a.append({"xthi": _pack(xthi, P), "xtlo": _pack(xtlo, P),
                          "rws": rws})
    res_a = run_bass_kernel_spmd(nc_a, in_maps_a, core_ids=list(range(N_CORES)))
    # unpack [P, M4, E] -> [SHARD, E] per core, concat to [S_TOT, E]
    dw = np.concatenate(
        [res_a.results[c]["dwp"].transpose(1, 0, 2).reshape(SHARD, E)
         for c in range(N_CORES)], axis=0)
    if _timings is not None:
        _timings["router_ns"] = res_a.exec_time_ns

    # ---- Host: build the dispatch (the all-to-all by expert) ----
    sel = dw > 0.0
    idx_list = [np.nonzero(sel[:, e])[0].astype(np.int32) for e in range(E)]
    counts = [len(ix) for ix in idx_list]
    cap = max(max(counts), 1)
    cap = ((cap + P - 1) // P) * P

    flat_bf = flat.astype(NP_BF16)
    wg_bf = np.asarray(w_gate, dtype=NP_BF16)
    wu_bf = np.asarray(w_up, dtype=NP_BF16)
    wd_bf = np.asarray(w_down, dtype=NP_BF16)

    in_maps_b = []
    for e in range(E):
        ix = idx_list[e]
        xg = np.zeros((cap, D), dtype=NP_BF16)
        xg[:len(ix)] = flat_bf[ix]
        wt = np.zeros(cap, dtype=np.float32)
        wt[:len(ix)] = dw[ix, e]
        in_maps_b.append({
            "xgT": _pack(np.ascontiguousarray(xg.T), P),
            "wg": _pack(wg_bf[e], P),
            "wu": _pack(wu_bf[e], P),
            "wd": _pack(wd_bf[e], P),
            "wtb": np.ascontiguousarray(np.broadcast_to(wt[None, :], (P, cap))),
        })

    # ---- Launch B: experts (expert-parallel) ----
    nc_b = _get_expert_nc(cap)
    res_b = run_bass_kernel_spmd(nc_b, in_maps_b, core_ids=list(range(N_CORES)))
    if _timings is not None:
        _timings["expert_ns"] = res_b.exec_time_ns

    # ---- Host: combine (scatter-add back, then unshard) ----
    out = np.zeros((S_TOT, D), dtype=np.float32)
    for e in range(E):
        ix = idx_list[e]
        if len(ix) == 0:
            continue
        ytp = res_b.results[e]["yt"]  # [P, KD, cap] bf16
        ytc = ytp.transpose(1, 0, 2).reshape(D, cap)[:, :len(ix)].astype(np.float32)
        out[ix] += ytc.T  # indices unique per expert
    return out.reshape(B, S, D)
